# revision 1
# baseline (speedup 1.0000x reference)
"""Trainium2 kernel for fused (div + 3D maxpool 2x2x2 + global avgpool + bias + channel-sum).

Reference computation (o: [N,C,D,H,W] f32, bias: [C,1,1,1] f32):
    x = o / 2
    x = maxpool3d(x, kernel=stride=(2,2,2))        # [N,C,16,32,32]
    x = x.mean(axis=(2,3,4))                        # [N,C]
    out[n] = sum_c (x[n,c] + bias[c])               # [N,1,1,1]

Algebraic simplification (max commutes with the positive scale 1/2):
    out[n] = (1/32768) * sum_{c,blocks} maxpool3d(o[n]) + sum_c bias[c]

Sharding: data-parallel over N across 8 cores (2 batches/core, no comm).

Per-core layout: o[n, c, 2*pd:2*pd+2, :, :] is a contiguous 8192-float chunk,
so the 32 MiB shard is viewed as [1024 rows, 8192] where every row holds one
(n, c, pd) d-pair slab: columns f and f+4096 are d-pair partners, and within
each 4096 half the layout is [h(64), w(64)].

Each 128-row tile streams in as four 1 MiB chunks (cols 0:2048, 2048:4096 =
d_in 0; 4096:6144, 6144:8192 = d_in 1). Per chunk on the vector engine:
  w-pair max  (even/odd columns)          [128,2048] -> [128,1024]
  h-pair max  (even/odd h rows)           [128,1024] -> [128, 512]
then the d-pair of chunk results merges via scalar_tensor_tensor
(max + fused running sum -> one accumulator column per pair).
Cross-partition finish: matmul with a SCALE-valued ones vector, two group
sums, bias add. The final row-tile is split into half-size chunks to shorten
the post-DMA compute tail.
"""

import numpy as np

N, C, D, H, W = 16, 32, 32, 64, 64
N_CORES = 8
N_PER_CORE = N // N_CORES          # 2
PD = D // 2                        # 16
ROWS = N_PER_CORE * C * PD         # 1024
FREE = 2 * H * W                   # 8192
P = 128                            # SBUF partitions
N_TILES = ROWS // P                # 8
Q = 2048                           # 1 MiB chunk columns
SCALE = 1.0 / (2.0 * PD * (H // 2) * (W // 2))  # 1/32768

_NC_CACHE = None


def _build_nc():
    import concourse.bacc as bacc
    import concourse.tile as tile
    import concourse.mybir as mybir

    f32 = mybir.dt.float32
    nc = bacc.Bacc("TRN2", target_bir_lowering=False, debug=False)

    o_in = nc.dram_tensor("o", [ROWS, FREE], f32, kind="ExternalInput")
    b_in = nc.dram_tensor("bias", [1, C], f32, kind="ExternalInput")
    out_d = nc.dram_tensor("out", [1, N_PER_CORE], f32, kind="ExternalOutput")

    with tile.TileContext(nc) as tc:
        with (
            tc.tile_pool(name="x", bufs=8) as xp,
            tc.tile_pool(name="w", bufs=6) as wp,
            tc.tile_pool(name="h", bufs=6) as hp,
            tc.tile_pool(name="m", bufs=2) as mp,
            tc.tile_pool(name="misc", bufs=1) as misc,
            tc.tile_pool(name="ps", bufs=1, space="PSUM") as pp,
        ):
            acc = misc.tile([P, 2 * N_TILES + 2], f32)
            ones = misc.tile([P, 1], f32)
            nc.vector.memset(ones[:], SCALE)
            # bias + final store ride the ACT HWDGE ring so the SP ring only
            # carries the bulk input stream
            bt = misc.tile([1, C], f32)
            nc.scalar.dma_start(bt[:], b_in[:])
            bsum = misc.tile([1, 1], f32)
            nc.vector.reduce_sum(bsum[:], bt[:], axis=mybir.AxisListType.X)

            def wmax(x, n):
                wt = wp.tile([P, n // 2], f32, tag="w")
                u = x[:].rearrange("p (q wi) -> p q wi", wi=2)
                nc.vector.tensor_max(wt[:], u[:, :, 0], u[:, :, 1])
                return wt

            def hmax(wt, n):
                ht = hp.tile([P, n // 4], f32, tag="h")
                v = wt[:].rearrange("p (h2 hi w2) -> p h2 hi w2", hi=2, w2=32)
                nc.vector.tensor_max(
                    ht[:].rearrange("p (h2 w2) -> p h2 w2", w2=32),
                    v[:, :, 0, :],
                    v[:, :, 1, :],
                )
                return ht

            def proc_chunk(rows, c0, n):
                x = xp.tile([P, n], f32, tag="x")
                nc.sync.dma_start(x[:], o_in[rows, c0 : c0 + n])
                return hmax(wmax(x, n), n)

            def stt(h0ap, h1, col, tag="m3"):
                m3 = mp.tile([P, h1.shape[1]], f32, tag=tag)
                nc.vector.scalar_tensor_tensor(
                    out=m3[:],
                    in0=h0ap,
                    scalar=0.0,
                    in1=h1[:],
                    op0=mybir.AluOpType.bypass,
                    op1=mybir.AluOpType.max,
                    accum_out=acc[:, col : col + 1],
                )

            col = 0
            boundary = None
            for t in range(N_TILES):
                rows = slice(P * t, P * (t + 1))
                if t == N_TILES // 2:
                    boundary = col
                h0 = proc_chunk(rows, 0, Q)
                h2_ = proc_chunk(rows, 2 * Q, Q)
                stt(h0[:], h2_, col)
                col += 1
                h1 = proc_chunk(rows, Q, Q)
                if t < N_TILES - 1:
                    h3 = proc_chunk(rows, 3 * Q, Q)
                    stt(h1[:], h3, col)
                    col += 1
                else:
                    # final d-pair in two half-chunks: shorter post-DMA tail
                    h3a = proc_chunk(rows, 3 * Q, Q // 2)
                    stt(h1[:, : Q // 8], h3a, col, tag="m3b")
                    col += 1
                    h3b = proc_chunk(rows, 3 * Q + Q // 2, Q // 2)
                    stt(h1[:, Q // 8 :], h3b, col, tag="m3b")
                    col += 1
            ncols = col

            # Cross-partition sum (scaled by the ones vector's SCALE value)
            ps = pp.tile([1, ncols], f32)
            nc.tensor.matmul(ps[:], ones[:], acc[:, :ncols], start=True, stop=True)
            res = misc.tile([1, N_PER_CORE], f32)
            nc.vector.reduce_sum(
                res[:, 0:1], ps[:, 0:boundary].unsqueeze(1), axis=mybir.AxisListType.X
            )
            nc.vector.reduce_sum(
                res[:, 1:2],
                ps[:, boundary:ncols].unsqueeze(1),
                axis=mybir.AxisListType.X,
            )
            fin = misc.tile([1, N_PER_CORE], f32)
            nc.vector.tensor_add(
                fin[:], res[:], bsum[:].to_broadcast((1, N_PER_CORE))
            )
            nc.scalar.dma_start(out_d[:], fin[:])

    nc.compile()
    return nc


_RUNNER_CACHE = None


def _build_runner(nc):
    """Jitted shard_map runner built once; per call only input upload +
    execution happen (run_bass_kernel_spmd re-traces jax on every call)."""
    import jax
    import numpy as _np
    from jax.sharding import Mesh, PartitionSpec, NamedSharding
    from concourse import bass2jax
    import concourse.mybir as mybir

    bass2jax.install_neuronx_cc_hook()
    partition_name = nc.partition_id_tensor.name if nc.partition_id_tensor else None
    in_names, out_names, out_avals, zero_outs = [], [], [], []
    for alloc in nc.m.functions[0].allocations:
        if not isinstance(alloc, mybir.MemoryLocationSet):
            continue
        name = alloc.memorylocations[0].name
        if alloc.kind == "ExternalInput":
            if name != partition_name:
                in_names.append(name)
        elif alloc.kind == "ExternalOutput":
            out_names.append(name)
            shape = tuple(alloc.tensor_shape)
            dtype = mybir.dt.np(alloc.dtype)
            out_avals.append(jax.core.ShapedArray(shape, dtype))
            zero_outs.append(_np.zeros(shape, dtype))
    n_params = len(in_names)
    n_outs = len(out_avals)
    all_in = list(in_names) + list(out_names)
    if partition_name is not None:
        all_in.append(partition_name)

    def _body(*args):
        operands = list(args)
        if partition_name is not None:
            operands.append(bass2jax.partition_id_tensor())
        return tuple(
            bass2jax._bass_exec_p.bind(
                *operands,
                out_avals=tuple(out_avals),
                in_names=tuple(all_in),
                out_names=tuple(out_names),
                lowering_input_output_aliases=(),
                sim_require_finite=True,
                sim_require_nnan=True,
                nc=nc,
            )
        )

    devices = jax.devices()[:N_CORES]
    mesh = Mesh(_np.asarray(devices), ("core",))
    n_tot = n_params + n_outs
    fn = jax.jit(
        jax.shard_map(
            _body,
            mesh=mesh,
            in_specs=(PartitionSpec("core"),) * n_tot,
            out_specs=(PartitionSpec("core"),) * n_outs,
            check_vma=False,
        ),
        donate_argnums=tuple(range(n_params, n_tot)),
        keep_unused=True,
    )
    sharding = NamedSharding(mesh, PartitionSpec("core"))

    def run(concat_inputs_by_name):
        dev_in = [
            jax.device_put(concat_inputs_by_name[nm], sharding) for nm in in_names
        ]
        zs = [
            jax.device_put(
                _np.zeros((N_CORES * z.shape[0],) + z.shape[1:], z.dtype), sharding
            )
            for z in zero_outs
        ]
        outs = fn(*dev_in, *zs)
        return {
            name: _np.asarray(outs[i]).reshape(N_CORES, *out_avals[i].shape)
            for i, name in enumerate(out_names)
        }

    return run


def kernel(o: np.ndarray, bias: np.ndarray) -> np.ndarray:
    global _NC_CACHE, _RUNNER_CACHE

    if _NC_CACHE is None:
        _NC_CACHE = _build_nc()
    nc = _NC_CACHE

    o = np.ascontiguousarray(o, dtype=np.float32)
    b2 = np.ascontiguousarray(bias, dtype=np.float32).reshape(1, C)
    o_rows = o.reshape(N_CORES * ROWS, FREE)  # shard k = rows [k*ROWS, (k+1)*ROWS)
    b_rep = np.broadcast_to(b2, (N_CORES, C)).reshape(N_CORES * 1, C)

    try:
        if _RUNNER_CACHE is None:
            _RUNNER_CACHE = _build_runner(nc)
        res = _RUNNER_CACHE({"o": o_rows, "bias": np.ascontiguousarray(b_rep)})
        out = res["out"].reshape(N_CORES * N_PER_CORE)
    except Exception:
        from concourse.bass_utils import run_bass_kernel_spmd

        in_maps = [
            {
                "o": o[N_PER_CORE * k : N_PER_CORE * (k + 1)].reshape(ROWS, FREE),
                "bias": b2,
            }
            for k in range(N_CORES)
        ]
        r = run_bass_kernel_spmd(nc, in_maps, core_ids=list(range(N_CORES)))
        out = np.concatenate(
            [r.results[k]["out"].reshape(N_PER_CORE) for k in range(N_CORES)]
        )
    return out.reshape(N, 1, 1, 1).astype(np.float32)

